# revision 11
# baseline (speedup 1.0000x reference)
"""Trainium2 Bass kernel for nn_DecoderTransformer (T=2048, D=2048, H=16, V=32000).

Strategy (8-way tensor parallel, full inputs in / full output out):
  - Host precomputes x = we[tok] + pe and ships xT pre-arranged as
    [128, dc, t]; no on-device gather or transposes.
  - Heads sharded 2-per-core. qT/kT computed per 512-col t-block; v is
    computed directly in [t, 2*hs] layout (xT chunk as the stationary
    operand), so attention needs no on-chip transposes at all.
  - Causal attention blockwise with softmax-without-max (sims range is
    ~[-11, 12], exp safe in f32); exp/v/heads carried in bf16.
  - AllGather(headsT bf16) per t-half -> catT; proj sharded over d_out
    (256 rows/core); resid = x_shard + sa_shard + pb computed BEFORE the
    second collective (host ships the core's x rows with pb pre-added),
    then AllGather(residT bf16) feeds fc directly.
  - Schedule: proj/AG2 for t-half 0 run right after its AllGather while
    attention for half 1 is still on the PE, so fc(half 0) starts as soon
    as attention ends; proj(half 1) is spliced into the fc stream.
  - fc sharded over vocab (4000 cols/core padded to 4096, bf16 weights,
    128-col chunks for fast weight load), outputs logitsT f32 [4000, 2048];
    host transposes and concatenates.
"""

import os

import numpy as np

T = 2048
D = 2048
H = 16
HS = 128
V = 32000
NCORES = 8
P = 128
DC = D // P            # 16 d chunks
TC = T // P            # 16 t chunks
HPC = H // NCORES      # 2 heads per core
VSH = V // NCORES      # 4000 vocab shard
NVC = 32               # vocab chunks of 128 (last holds 16 real rows)
DOS = D // NCORES      # 256 d_out shard rows

_CACHE = {}


def _build():
    import concourse.bass as bass  # noqa: F401
    import concourse.tile as tile
    from concourse import bacc, mybir

    f32 = mybir.dt.float32
    f32r = mybir.dt.float32r
    bf16 = mybir.dt.bfloat16
    EXP = mybir.ActivationFunctionType.Exp
    COPY = mybir.ActivationFunctionType.Copy
    RG = [list(range(NCORES))]

    nc = bacc.Bacc("TRN2", target_bir_lowering=False, debug=False,
                   num_devices=NCORES, num_swdge_queues=4)

    xt = nc.dram_tensor("xt", [P, DC * T], f32, kind="ExternalInput")
    wq = nc.dram_tensor("wq", [P, DC * 256], f32, kind="ExternalInput")
    wk = nc.dram_tensor("wk", [P, DC * 256], f32, kind="ExternalInput")
    wv = nc.dram_tensor("wv", [P, DC * 256], f32, kind="ExternalInput")
    pw = nc.dram_tensor("pw", [P, DC * 256], bf16, kind="ExternalInput")
    xpb = nc.dram_tensor("xpb", [P, 2 * T], f32, kind="ExternalInput")
    fw = nc.dram_tensor("fw", [NVC * P, DC * P], bf16, kind="ExternalInput")
    fb = nc.dram_tensor("fb", [P, NVC], f32, kind="ExternalInput")
    outT = nc.dram_tensor("outT", [VSH, T], f32, kind="ExternalOutput")

    with tile.TileContext(nc) as tc:
        dram = tc.alloc_tile_pool(name="dram", bufs=1, space="DRAM")
        pconst = tc.alloc_tile_pool(name="pconst", bufs=1)

        ag1_in = [dram.tile([HPC * HS, 1024], bf16, name=f"ag1_in{h}")
                  for h in range(2)]
        ag1_out = [dram.tile([D, 1024], bf16, name=f"ag1_out{h}",
                             addr_space="Shared") for h in range(2)]
        ag2_in = [dram.tile([DOS, 1024], bf16, name=f"ag2_in{h}")
                  for h in range(2)]
        ag2_out = [dram.tile([D, 1024], bf16, name=f"ag2_out{h}",
                             addr_space="Shared") for h in range(2)]

        ones_bf = pconst.tile([P, 1], bf16, name="ones_bf")
        ones_row_f = pconst.tile([1, P], f32, name="ones_row_f")
        ones_row = pconst.tile([1, P], f32r, name="ones_row")
        fb_s = pconst.tile([P, NVC], f32, name="fb_s")
        maskbig = pconst.tile([P, 896], f32, name="maskbig")

        with tc.tile_pool(name="pm", bufs=1) as pm, \
             tc.tile_pool(name="pcat", bufs=1) as pcat:
            qT = [pm.tile([P, T], f32r, name=f"qT{h}") for h in range(HPC)]
            kT = [pm.tile([P, T], f32r, name=f"kT{h}") for h in range(HPC)]
            v_all = pm.tile([P, TC, HPC * HS], bf16, name="v_all")
            xpb_s = pm.tile([P, 2, T], f32, name="xpb_s")
            pw_t = pm.tile([P, DC, 2 * P], bf16, name="pw_t")

            def emit_proj(h4, pspool, pstag):
                for tb2 in range(2):
                    ps_o = [pspool.tile([P, 512], f32, tag=pstag,
                                        name=f"pj{h4}{tb2}_{o}")
                            for o in range(2)]
                    for dcg in range(4):
                        cat_t = pcat.tile([P, 4, 512], bf16, tag="catT",
                                          bufs=3, name=f"cat{h4}{tb2}_{dcg}")
                        nc.gpsimd.dma_start(
                            out=cat_t[:],
                            in_=ag1_out[h4][dcg * 4 * P:(dcg + 1) * 4 * P,
                                            tb2 * 512:(tb2 + 1) * 512]
                            .rearrange("(c p) t -> p c t", p=P))
                        for dci in range(4):
                            dc = dcg * 4 + dci
                            for o in range(2):
                                nc.tensor.matmul(
                                    ps_o[o][:],
                                    pw_t[:, dc, o * P:(o + 1) * P],
                                    cat_t[:, dci, :],
                                    start=(dc == 0), stop=(dc == DC - 1))
                    for o in range(2):
                        res_t = pcat.tile([P, 512], bf16, tag="res",
                                          bufs=3, name=f"res{h4}{tb2}_{o}")
                        nc.vector.tensor_add(
                            res_t[:], ps_o[o][:],
                            xpb_s[:, o, h4 * 1024 + tb2 * 512:
                                  h4 * 1024 + (tb2 + 1) * 512])
                        nc.scalar.dma_start(
                            out=ag2_in[h4][o * P:(o + 1) * P,
                                           tb2 * 512:(tb2 + 1) * 512],
                            in_=res_t[:])
                nc.gpsimd.collective_compute(
                    "AllGather", mybir.AluOpType.bypass, replica_groups=RG,
                    ins=[ag2_in[h4][:]], outs=[ag2_out[h4][:]])

            # ---- Phases A (qkv) + B (attention) + proj(0) ----
            with tc.tile_pool(name="px", bufs=1) as px, \
                 tc.tile_pool(name="pw8", bufs=2) as pww, \
                 tc.tile_pool(name="pat", bufs=1) as pat, \
                 tc.tile_pool(name="psA", bufs=4, space="PSUM") as psA, \
                 tc.tile_pool(name="psV", bufs=2, space="PSUM") as psV, \
                 tc.tile_pool(name="psC", bufs=1, space="PSUM") as psC, \
                 tc.tile_pool(name="psB", bufs=1, space="PSUM") as psB:
                w_tiles = {}
                xt_tiles = {}

                def emit_loads(h4):
                    # weight streams lead the scalar (Act) HWDGE queue;
                    # xt stream is split across sync (even dc) and gpsimd
                    # (odd dc) queues so the PE is fed at 2x one queue's
                    # bandwidth.
                    for wdram in (wq, wk, wv):
                        w_t = pww.tile([P, DC, HPC * HS], f32r, tag="w",
                                       bufs=2, name=f"w{wdram.name}_{h4}")
                        nc.scalar.dma_start(
                            out=w_t[:],
                            in_=wdram[:].bitcast(f32r)
                            .rearrange("p (dc j) -> p dc j", dc=DC))
                        w_tiles[(wdram.name, h4)] = w_t
                    xt_h = px.tile([P, DC, 1024], f32r, tag="xt", bufs=1,
                                   name=f"xt{h4}")
                    for dc in range(DC):
                        eng = nc.sync if dc % 2 == 0 else nc.gpsimd
                        eng.dma_start(
                            out=xt_h[:, dc, :],
                            in_=xt[:][:, dc * T + h4 * 1024:
                                      dc * T + (h4 + 1) * 1024].bitcast(f32r))
                    xt_tiles[h4] = xt_h

                def emit_qkv(h4):
                    xt_h = xt_tiles[h4]
                    for wname, outs in (("wq", qT), ("wk", kT)):
                        w_t = w_tiles[(wname, h4)]
                        for b2 in range(2):
                            ps = {h: psA.tile([P, 512], f32, tag="qk",
                                              name=f"ps{wname}{h}_{h4}{b2}")
                                  for h in range(HPC)}
                            for dc in range(DC):
                                for h in range(HPC):
                                    nc.tensor.matmul(
                                        ps[h][:],
                                        w_t[:, dc, h * HS:(h + 1) * HS],
                                        xt_h[:, dc, b2 * 512:(b2 + 1) * 512],
                                        start=(dc == 0), stop=(dc == DC - 1))
                            for h in range(HPC):
                                nc.vector.tensor_copy(
                                    outs[h][:, (2 * h4 + b2) * 512:
                                            (2 * h4 + b2 + 1) * 512],
                                    ps[h][:])
                    w_t = w_tiles[("wv", h4)]
                    for b2 in range(2):
                        vps = [psV.tile([P, 2, 256], f32, tag="v",
                                        name=f"psv{h4}{b2}{j}")
                               for j in range(2)]
                        # each sub-group's dc loop runs to completion before
                        # the bank's other group starts: a start=True matmul
                        # clears has_written for the WHOLE psum bank, so
                        # interleaved sub-bank accumulation groups corrupt
                        # each other's first contribution.
                        for tc4 in range(4):
                            for dc in range(DC):
                                nc.tensor.matmul(
                                    vps[tc4 // 2][:, tc4 % 2, :],
                                    xt_h[:, dc, b2 * 512 + tc4 * P:
                                         b2 * 512 + (tc4 + 1) * P],
                                    w_t[:, dc, :],
                                    start=(dc == 0), stop=(dc == DC - 1))
                        for tc4 in range(4):
                            nc.vector.tensor_copy(
                                v_all[:, h4 * 8 + b2 * 4 + tc4, :],
                                vps[tc4 // 2][:, tc4 % 2, :])

                def emit_attn(h4):
                    for g2 in range(2):
                        g = 2 * h4 + g2
                        nsc = 4 * g + 4
                        for h in range(HPC):
                            expT = pat.tile([P, TC, 512], bf16, tag="expT",
                                            bufs=1, name=f"expT{h}_{g}")
                            cs_ps = psC.tile([1, 512], f32, tag="cs",
                                             name=f"cs{h}_{g}")
                            for c in range(nsc):
                                s_ps = psA.tile([P, 512], f32, tag="qk",
                                                name=f"sims{h}_{g}_{c}")
                                nc.tensor.matmul(
                                    s_ps[:], kT[h][:, c * P:(c + 1) * P],
                                    qT[h][:, g * 512:(g + 1) * 512],
                                    start=True, stop=True)
                                if c >= 4 * g:
                                    m = c - 4 * g
                                    nc.vector.tensor_add(
                                        s_ps[:], s_ps[:],
                                        maskbig[:, 384 - 128 * m:
                                                896 - 128 * m])
                                nc.scalar.activation(out=expT[:, c, :],
                                                     in_=s_ps[:], func=EXP)
                            for c in range(nsc):
                                nc.tensor.matmul(cs_ps[:], ones_bf[:],
                                                 expT[:, c, :],
                                                 start=(c == 0),
                                                 stop=(c == nsc - 1))
                            av_ps = psV.tile([P, 512], f32, tag="v",
                                             name=f"av{h}_{g}")
                            for c in range(nsc):
                                nc.tensor.matmul(
                                    av_ps[:],
                                    v_all[:, c, h * HS:(h + 1) * HS],
                                    expT[:, c, :],
                                    start=(c == 0), stop=(c == nsc - 1))
                            recip = pat.tile([1, 512], f32, tag="recip",
                                             bufs=1, name=f"rc{h}_{g}")
                            nc.vector.reciprocal(recip[:], cs_ps[:])
                            recip_r = pat.tile([1, 512], f32r, tag="recip_r",
                                               bufs=1, name=f"rcr{h}_{g}")
                            nc.vector.tensor_copy(recip_r[:], recip[:])
                            bc_ps = psB.tile([P, 512], f32, tag="bc",
                                             name=f"bc{h}_{g}")
                            nc.tensor.matmul(bc_ps[:], ones_row[:],
                                             recip_r[:],
                                             start=True, stop=True)
                            bc_s = pat.tile([P, 512], f32, tag="bc_s",
                                            bufs=2, name=f"bcs{h}_{g}")
                            nc.scalar.activation(out=bc_s[:], in_=bc_ps[:],
                                                 func=COPY)
                            stage = pat.tile([P, 512], bf16, tag="stage",
                                             bufs=2, name=f"st{h}_{g}")
                            nc.vector.tensor_mul(stage[:], av_ps[:], bc_s[:])
                            nc.scalar.dma_start(
                                out=ag1_in[h4][h * HS:(h + 1) * HS,
                                               g2 * 512:(g2 + 1) * 512],
                                in_=stage[:])
                    nc.gpsimd.collective_compute(
                        "AllGather", mybir.AluOpType.bypass,
                        replica_groups=RG,
                        ins=[ag1_in[h4][:]], outs=[ag1_out[h4][:]])

                emit_loads(0)
                # small consts after the weight streams on their queues
                nc.vector.memset(ones_bf[:], 1.0)
                nc.vector.memset(ones_row_f[:], 1.0)
                nc.vector.tensor_copy(ones_row[:], ones_row_f[:])
                nc.gpsimd.memset(maskbig[:], 0.0)
                nc.gpsimd.affine_select(
                    out=maskbig[:], in_=maskbig[:],
                    compare_op=mybir.AluOpType.is_ge, fill=-40.0,
                    base=-384, pattern=[[1, 896]], channel_multiplier=-1,
                )
                nc.scalar.dma_start(
                    out=xpb_s[:],
                    in_=xpb[:].rearrange("p (o t) -> p o t", o=2))
                nc.scalar.dma_start(
                    out=pw_t[:],
                    in_=pw[:].rearrange("p (dc j) -> p dc j", dc=DC))
                nc.scalar.dma_start(out=fb_s[:], in_=fb[:])

                emit_qkv(0)
                emit_loads(1)
                emit_attn(0)
                emit_qkv(1)
                emit_proj(0, psA, "qk")
                emit_attn(1)

            # ---- Phase D: fc over vocab, proj(1) spliced in ----
            with tc.tile_pool(name="pfc", bufs=1) as pfc, \
                 tc.tile_pool(name="psF", bufs=8, space="PSUM") as psF:
                rt = {}

                def emit_rt(h4):
                    rt_h = pfc.tile([P, DC, 1024], bf16, tag="rt", bufs=2,
                                    name=f"rt{h4}")
                    for dc in range(DC):
                        nc.sync.dma_start(
                            out=rt_h[:, dc, :],
                            in_=ag2_out[h4][dc * P:(dc + 1) * P, :])
                    rt[h4] = rt_h

                def emit_fc(h4, vcs):
                    for vc in vcs:
                        fw_t = pfc.tile([P, DC, P], bf16, tag="fw_t",
                                        bufs=6, name=f"fw{h4}_{vc}")
                        nc.sync.dma_start(
                            out=fw_t[:],
                            in_=fw[:][vc * P:(vc + 1) * P, :]
                            .rearrange("p (dc j) -> p dc j", dc=DC))
                        psf = {tb2: psF.tile([P, 512], f32, tag="fc",
                                             name=f"fc{h4}_{vc}_{tb2}")
                               for tb2 in range(2)}
                        for dc in range(DC):
                            for tb2 in range(2):
                                nc.tensor.matmul(
                                    psf[tb2][:], fw_t[:, dc, :],
                                    rt[h4][:, dc, tb2 * 512:(tb2 + 1) * 512],
                                    start=(dc == 0), stop=(dc == DC - 1))
                        rows = P if vc < NVC - 1 else VSH - (NVC - 1) * P
                        for tb2 in range(2):
                            ev = pfc.tile([P, 512], f32, tag="fc_ev",
                                          bufs=4, name=f"fcev{h4}_{vc}_{tb2}")
                            nc.vector.tensor_scalar_add(ev[:], psf[tb2][:],
                                                        fb_s[:, vc:vc + 1])
                            nc.scalar.dma_start(
                                out=outT[:][vc * P:vc * P + rows,
                                            h4 * 1024 + tb2 * 512:
                                            h4 * 1024 + (tb2 + 1) * 512],
                                in_=ev[:rows, :])

                emit_rt(0)
                emit_fc(0, range(0, 8))
                emit_proj(1, psF, "fc")
                emit_fc(0, range(8, NVC))
                emit_rt(1)
                emit_fc(1, range(NVC))

        dram.release()
        pconst.release()

    nc.compile()
    return nc


def _get_nc():
    if "nc" not in _CACHE:
        _CACHE["nc"] = _build()
    return _CACHE["nc"]


def _arr_pdc(a):
    """[D, N] -> [128, DC*N] with row d = dc*128 + p."""
    d, n = a.shape
    return np.ascontiguousarray(
        a.reshape(DC, P, n).transpose(1, 0, 2).reshape(P, DC * n))


def kernel(token_ids, we, pe, Wq, Wk, Wv, proj_w, proj_b, fc_w, fc_b):
    import ml_dtypes
    from concourse.bass_utils import run_bass_kernel_spmd

    bf16 = ml_dtypes.bfloat16

    tok = np.asarray(token_ids).astype(np.int64)
    we = np.asarray(we, dtype=np.float32)
    pe = np.asarray(pe, dtype=np.float32)[:T]
    Wq = np.asarray(Wq, dtype=np.float32)
    Wk = np.asarray(Wk, dtype=np.float32)
    Wv = np.asarray(Wv, dtype=np.float32)
    proj_w = np.asarray(proj_w, dtype=np.float32)
    proj_b = np.asarray(proj_b, dtype=np.float32)
    fc_w = np.asarray(fc_w, dtype=np.float32)
    fc_b = np.asarray(fc_b, dtype=np.float32)

    x = we[tok] + pe                      # [T, D] f32, on host
    xT = np.ascontiguousarray(x.T)        # [D, T]
    xt_arr = _arr_pdc(xT)                 # [128, DC*T]

    scale = np.float32(1.0 / np.sqrt(HS))
    in_maps = []
    for i in range(NCORES):
        h0 = HPC * i
        wq_i = _arr_pdc(np.concatenate(
            [Wq[h0 + j] for j in range(HPC)], axis=1) * scale)
        wk_i = _arr_pdc(np.concatenate(
            [Wk[h0 + j] for j in range(HPC)], axis=1))
        wv_i = _arr_pdc(np.concatenate(
            [Wv[h0 + j] for j in range(HPC)], axis=1))
        pw_i = _arr_pdc(proj_w[:, DOS * i:DOS * (i + 1)]).astype(bf16)
        # core's x rows + proj bias, transposed: [256, T] -> [128, 2*T]
        xpb_i = (x[:, DOS * i:DOS * (i + 1)] +
                 proj_b[DOS * i:DOS * (i + 1)]).T
        xpb_i = np.ascontiguousarray(
            xpb_i.reshape(2, P, T).transpose(1, 0, 2).reshape(P, 2 * T))
        # fc shard padded 4000 -> 4096 vocab cols, chunked [vc][p][dc][j]
        fw_pad = np.zeros((D, NVC * P), dtype=np.float32)
        fw_pad[:, :VSH] = fc_w[:, VSH * i:VSH * (i + 1)]
        fw_i = np.ascontiguousarray(
            fw_pad.reshape(DC, P, NVC, P).transpose(2, 1, 0, 3)
            .reshape(NVC * P, DC * P)).astype(bf16)
        fb_pad = np.zeros(NVC * P, dtype=np.float32)
        fb_pad[:VSH] = fc_b[VSH * i:VSH * (i + 1)]
        fb_i = np.ascontiguousarray(fb_pad.reshape(NVC, P).T)
        in_maps.append({
            "xt": xt_arr, "wq": wq_i, "wk": wk_i, "wv": wv_i,
            "pw": pw_i, "xpb": xpb_i, "fw": fw_i, "fb": fb_i,
        })

    nc = _get_nc()
    trace = bool(int(os.environ.get("BASSKERNEL_TRACE", "0")))
    res = run_bass_kernel_spmd(nc, in_maps, core_ids=list(range(NCORES)),
                               trace=trace)
    if trace and res.exec_time_ns is not None:
        print(f"HW exec time: {res.exec_time_ns} ns")
        if res.instructions_and_trace is not None:
            print(f"Trace: {res.instructions_and_trace[1]}")

    out = np.empty((T, V), dtype=np.float32)
    for i in range(NCORES):
        out[:, VSH * i:VSH * (i + 1)] = res.results[i]["outT"].T
    return out


# revision 12
# speedup vs baseline: 1.0821x; 1.0821x over previous
"""Trainium2 Bass kernel for nn_DecoderTransformer (T=2048, D=2048, H=16, V=32000).

Strategy (8-way tensor parallel, full inputs in / full output out):
  - Host precomputes x = we[tok] + pe and ships xT pre-arranged as
    [128, dc, t]; no on-device gather or transposes.
  - Heads sharded 2-per-core. qT/kT computed per 512-col t-block; v is
    computed directly in [t, 2*hs] layout (xT chunk as the stationary
    operand), so attention needs no on-chip transposes at all.
  - Causal attention blockwise with softmax-without-max (sims range is
    ~[-11, 12], exp safe in f32); exp/v/heads carried in bf16.
  - AllGather(headsT bf16) per t-half -> catT; proj sharded over d_out
    (256 rows/core); resid = x_shard + sa_shard + pb computed BEFORE the
    second collective (host ships the core's x rows with pb pre-added),
    then AllGather(residT bf16) feeds fc directly.
  - Schedule: proj/AG2 for t-half 0 run right after its AllGather while
    attention for half 1 is still on the PE, so fc(half 0) starts as soon
    as attention ends; proj(half 1) is spliced into the fc stream.
  - fc sharded over vocab (4000 cols/core padded to 4096, bf16 weights,
    128-col chunks for fast weight load), outputs logitsT f32 [4000, 2048];
    host transposes and concatenates.
"""

import os

import numpy as np

T = 2048
D = 2048
H = 16
HS = 128
V = 32000
NCORES = 8
P = 128
DC = D // P            # 16 d chunks
TC = T // P            # 16 t chunks
HPC = H // NCORES      # 2 heads per core
VSH = V // NCORES      # 4000 vocab shard
NVC = 32               # vocab chunks of 128 (last holds 16 real rows)
DOS = D // NCORES      # 256 d_out shard rows

_CACHE = {}


def _build():
    import concourse.bass as bass  # noqa: F401
    import concourse.tile as tile
    from concourse import bacc, mybir

    f32 = mybir.dt.float32
    f32r = mybir.dt.float32r
    bf16 = mybir.dt.bfloat16
    EXP = mybir.ActivationFunctionType.Exp
    COPY = mybir.ActivationFunctionType.Copy
    RG = [list(range(NCORES))]

    nc = bacc.Bacc("TRN2", target_bir_lowering=False, debug=False,
                   num_devices=NCORES, num_swdge_queues=4)

    xt = nc.dram_tensor("xt", [P, DC * T], f32, kind="ExternalInput")
    wq = nc.dram_tensor("wq", [P, DC * 256], f32, kind="ExternalInput")
    wk = nc.dram_tensor("wk", [P, DC * 256], f32, kind="ExternalInput")
    wv = nc.dram_tensor("wv", [P, DC * 256], f32, kind="ExternalInput")
    pw = nc.dram_tensor("pw", [P, DC * 256], bf16, kind="ExternalInput")
    xpb = nc.dram_tensor("xpb", [P, 2 * T], f32, kind="ExternalInput")
    fw = nc.dram_tensor("fw", [NVC * P, DC * P], bf16, kind="ExternalInput")
    fb = nc.dram_tensor("fb", [P, NVC], f32, kind="ExternalInput")
    outT = nc.dram_tensor("outT", [VSH, T], f32, kind="ExternalOutput")

    with tile.TileContext(nc) as tc:
        dram = tc.alloc_tile_pool(name="dram", bufs=1, space="DRAM")
        pconst = tc.alloc_tile_pool(name="pconst", bufs=1)

        ag1_in = [dram.tile([HPC * HS, 1024], bf16, name=f"ag1_in{h}")
                  for h in range(2)]
        ag1_out = [dram.tile([D, 1024], bf16, name=f"ag1_out{h}",
                             addr_space="Shared") for h in range(2)]
        ag2_in = [dram.tile([DOS, 1024], bf16, name=f"ag2_in{h}")
                  for h in range(2)]
        ag2_out = [dram.tile([D, 1024], bf16, name=f"ag2_out{h}",
                             addr_space="Shared") for h in range(2)]

        ones_bf = pconst.tile([P, 1], bf16, name="ones_bf")
        ones_row_f = pconst.tile([1, P], f32, name="ones_row_f")
        ones_row = pconst.tile([1, P], f32r, name="ones_row")
        fb_s = pconst.tile([P, NVC], f32, name="fb_s")
        maskbig = pconst.tile([P, 896], f32, name="maskbig")

        with tc.tile_pool(name="pm", bufs=1) as pm, \
             tc.tile_pool(name="pcat", bufs=1) as pcat:
            qT = [pm.tile([P, T], f32r, name=f"qT{h}") for h in range(HPC)]
            kT = [pm.tile([P, T], f32r, name=f"kT{h}") for h in range(HPC)]
            v_all = pm.tile([P, TC, HPC * HS], bf16, name="v_all")
            xpb_s = pm.tile([P, 2, T], f32, name="xpb_s")
            pw_t = pm.tile([P, DC, 2 * P], bf16, name="pw_t")

            def emit_proj(h4, pspool, pstag):
                for tb2 in range(2):
                    ps_o = [pspool.tile([P, 512], f32, tag=pstag,
                                        name=f"pj{h4}{tb2}_{o}")
                            for o in range(2)]
                    for dcg in range(4):
                        cat_t = pcat.tile([P, 4, 512], bf16, tag="catT",
                                          bufs=3, name=f"cat{h4}{tb2}_{dcg}")
                        nc.scalar.dma_start(
                            out=cat_t[:],
                            in_=ag1_out[h4][dcg * 4 * P:(dcg + 1) * 4 * P,
                                            tb2 * 512:(tb2 + 1) * 512]
                            .rearrange("(c p) t -> p c t", p=P))
                        for dci in range(4):
                            dc = dcg * 4 + dci
                            for o in range(2):
                                nc.tensor.matmul(
                                    ps_o[o][:],
                                    pw_t[:, dc, o * P:(o + 1) * P],
                                    cat_t[:, dci, :],
                                    start=(dc == 0), stop=(dc == DC - 1))
                    for o in range(2):
                        res_t = pcat.tile([P, 512], bf16, tag="res",
                                          bufs=3, name=f"res{h4}{tb2}_{o}")
                        nc.vector.tensor_add(
                            res_t[:], ps_o[o][:],
                            xpb_s[:, o, h4 * 1024 + tb2 * 512:
                                  h4 * 1024 + (tb2 + 1) * 512])
                        nc.scalar.dma_start(
                            out=ag2_in[h4][o * P:(o + 1) * P,
                                           tb2 * 512:(tb2 + 1) * 512],
                            in_=res_t[:])
                nc.gpsimd.collective_compute(
                    "AllGather", mybir.AluOpType.bypass, replica_groups=RG,
                    ins=[ag2_in[h4][:]], outs=[ag2_out[h4][:]])

            # ---- Phases A (qkv) + B (attention) + proj(0) ----
            with tc.tile_pool(name="px", bufs=1) as px, \
                 tc.tile_pool(name="pw8", bufs=2) as pww, \
                 tc.tile_pool(name="pat", bufs=1) as pat, \
                 tc.tile_pool(name="psA", bufs=4, space="PSUM") as psA, \
                 tc.tile_pool(name="psV", bufs=2, space="PSUM") as psV, \
                 tc.tile_pool(name="psC", bufs=1, space="PSUM") as psC, \
                 tc.tile_pool(name="psB", bufs=1, space="PSUM") as psB:
                w_tiles = {}
                xt_tiles = {}

                def emit_loads(h4):
                    # weight streams lead the scalar (Act) HWDGE queue;
                    # xt stream is split across sync (even dc) and gpsimd
                    # (odd dc) queues so the PE is fed at 2x one queue's
                    # bandwidth.
                    for wdram in (wq, wk, wv):
                        w_t = pww.tile([P, DC, HPC * HS], f32r, tag="w",
                                       bufs=2, name=f"w{wdram.name}_{h4}")
                        nc.scalar.dma_start(
                            out=w_t[:],
                            in_=wdram[:].bitcast(f32r)
                            .rearrange("p (dc j) -> p dc j", dc=DC))
                        w_tiles[(wdram.name, h4)] = w_t
                    xt_h = px.tile([P, DC, 1024], f32r, tag="xt", bufs=1,
                                   name=f"xt{h4}")
                    for dc in range(DC):
                        nc.sync.dma_start(
                            out=xt_h[:, dc, :],
                            in_=xt[:][:, dc * T + h4 * 1024:
                                      dc * T + (h4 + 1) * 1024].bitcast(f32r))
                    xt_tiles[h4] = xt_h

                def emit_qkv(h4):
                    xt_h = xt_tiles[h4]
                    for wname, outs in (("wq", qT), ("wk", kT)):
                        w_t = w_tiles[(wname, h4)]
                        for b2 in range(2):
                            ps = {h: psA.tile([P, 512], f32, tag="qk",
                                              name=f"ps{wname}{h}_{h4}{b2}")
                                  for h in range(HPC)}
                            for dc in range(DC):
                                for h in range(HPC):
                                    nc.tensor.matmul(
                                        ps[h][:],
                                        w_t[:, dc, h * HS:(h + 1) * HS],
                                        xt_h[:, dc, b2 * 512:(b2 + 1) * 512],
                                        start=(dc == 0), stop=(dc == DC - 1))
                            for h in range(HPC):
                                nc.vector.tensor_copy(
                                    outs[h][:, (2 * h4 + b2) * 512:
                                            (2 * h4 + b2 + 1) * 512],
                                    ps[h][:])
                    w_t = w_tiles[("wv", h4)]
                    for b2 in range(2):
                        vps = [psV.tile([P, 2, 256], f32, tag="v",
                                        name=f"psv{h4}{b2}{j}")
                               for j in range(2)]
                        # each sub-group's dc loop runs to completion before
                        # the bank's other group starts: a start=True matmul
                        # clears has_written for the WHOLE psum bank, so
                        # interleaved sub-bank accumulation groups corrupt
                        # each other's first contribution.
                        for tc4 in range(4):
                            for dc in range(DC):
                                nc.tensor.matmul(
                                    vps[tc4 // 2][:, tc4 % 2, :],
                                    xt_h[:, dc, b2 * 512 + tc4 * P:
                                         b2 * 512 + (tc4 + 1) * P],
                                    w_t[:, dc, :],
                                    start=(dc == 0), stop=(dc == DC - 1))
                        for tc4 in range(4):
                            nc.vector.tensor_copy(
                                v_all[:, h4 * 8 + b2 * 4 + tc4, :],
                                vps[tc4 // 2][:, tc4 % 2, :])

                def emit_attn(h4):
                    for g2 in range(2):
                        g = 2 * h4 + g2
                        nsc = 4 * g + 4
                        for h in range(HPC):
                            expT = pat.tile([P, TC, 512], bf16, tag="expT",
                                            bufs=1, name=f"expT{h}_{g}")
                            cs_ps = psC.tile([1, 512], f32, tag="cs",
                                             name=f"cs{h}_{g}")
                            for c in range(nsc):
                                s_ps = psA.tile([P, 512], f32, tag="qk",
                                                name=f"sims{h}_{g}_{c}")
                                nc.tensor.matmul(
                                    s_ps[:], kT[h][:, c * P:(c + 1) * P],
                                    qT[h][:, g * 512:(g + 1) * 512],
                                    start=True, stop=True)
                                if c >= 4 * g:
                                    m = c - 4 * g
                                    nc.vector.tensor_add(
                                        s_ps[:], s_ps[:],
                                        maskbig[:, 384 - 128 * m:
                                                896 - 128 * m])
                                nc.scalar.activation(out=expT[:, c, :],
                                                     in_=s_ps[:], func=EXP)
                            for c in range(nsc):
                                nc.tensor.matmul(cs_ps[:], ones_bf[:],
                                                 expT[:, c, :],
                                                 start=(c == 0),
                                                 stop=(c == nsc - 1))
                            av_ps = psV.tile([P, 512], f32, tag="v",
                                             name=f"av{h}_{g}")
                            for c in range(nsc):
                                nc.tensor.matmul(
                                    av_ps[:],
                                    v_all[:, c, h * HS:(h + 1) * HS],
                                    expT[:, c, :],
                                    start=(c == 0), stop=(c == nsc - 1))
                            recip = pat.tile([1, 512], f32, tag="recip",
                                             bufs=1, name=f"rc{h}_{g}")
                            nc.vector.reciprocal(recip[:], cs_ps[:])
                            recip_r = pat.tile([1, 512], f32r, tag="recip_r",
                                               bufs=1, name=f"rcr{h}_{g}")
                            nc.vector.tensor_copy(recip_r[:], recip[:])
                            bc_ps = psB.tile([P, 512], f32, tag="bc",
                                             name=f"bc{h}_{g}")
                            nc.tensor.matmul(bc_ps[:], ones_row[:],
                                             recip_r[:],
                                             start=True, stop=True)
                            bc_s = pat.tile([P, 512], f32, tag="bc_s",
                                            bufs=2, name=f"bcs{h}_{g}")
                            nc.scalar.activation(out=bc_s[:], in_=bc_ps[:],
                                                 func=COPY)
                            stage = pat.tile([P, 512], bf16, tag="stage",
                                             bufs=2, name=f"st{h}_{g}")
                            nc.vector.tensor_mul(stage[:], av_ps[:], bc_s[:])
                            nc.scalar.dma_start(
                                out=ag1_in[h4][h * HS:(h + 1) * HS,
                                               g2 * 512:(g2 + 1) * 512],
                                in_=stage[:])
                    nc.gpsimd.collective_compute(
                        "AllGather", mybir.AluOpType.bypass,
                        replica_groups=RG,
                        ins=[ag1_in[h4][:]], outs=[ag1_out[h4][:]])

                emit_loads(0)
                # small consts after the weight streams on their queues
                nc.vector.memset(ones_bf[:], 1.0)
                nc.vector.memset(ones_row_f[:], 1.0)
                nc.vector.tensor_copy(ones_row[:], ones_row_f[:])
                nc.gpsimd.memset(maskbig[:], 0.0)
                nc.gpsimd.affine_select(
                    out=maskbig[:], in_=maskbig[:],
                    compare_op=mybir.AluOpType.is_ge, fill=-40.0,
                    base=-384, pattern=[[1, 896]], channel_multiplier=-1,
                )
                nc.scalar.dma_start(
                    out=xpb_s[:],
                    in_=xpb[:].rearrange("p (o t) -> p o t", o=2))
                nc.scalar.dma_start(
                    out=pw_t[:],
                    in_=pw[:].rearrange("p (dc j) -> p dc j", dc=DC))
                nc.scalar.dma_start(out=fb_s[:], in_=fb[:])

                emit_qkv(0)
                emit_loads(1)
                emit_attn(0)
                emit_qkv(1)
                emit_proj(0, psA, "qk")
                emit_attn(1)

            # ---- Phase D: fc over vocab, proj(1) spliced in ----
            with tc.tile_pool(name="pfc", bufs=1) as pfc, \
                 tc.tile_pool(name="psF", bufs=8, space="PSUM") as psF:
                rt = {}

                def emit_rt(h4):
                    rt_h = pfc.tile([P, DC, 1024], bf16, tag="rt", bufs=2,
                                    name=f"rt{h4}")
                    for dc in range(DC):
                        nc.sync.dma_start(
                            out=rt_h[:, dc, :],
                            in_=ag2_out[h4][dc * P:(dc + 1) * P, :])
                    rt[h4] = rt_h

                def load_fw(h4, vc):
                    fw_t = pfc.tile([P, DC, P], bf16, tag="fw_t",
                                    bufs=6, name=f"fw{h4}_{vc}")
                    nc.sync.dma_start(
                        out=fw_t[:],
                        in_=fw[:][vc * P:(vc + 1) * P, :]
                        .rearrange("p (dc j) -> p dc j", dc=DC))
                    return fw_t

                def emit_fc(h4, vcs, fw_pre=None):
                    for vc in vcs:
                        fw_t = (fw_pre.pop(vc, None) if fw_pre else None) \
                            or load_fw(h4, vc)
                        psf = {tb2: psF.tile([P, 512], f32, tag="fc",
                                             name=f"fc{h4}_{vc}_{tb2}")
                               for tb2 in range(2)}
                        for dc in range(DC):
                            for tb2 in range(2):
                                nc.tensor.matmul(
                                    psf[tb2][:], fw_t[:, dc, :],
                                    rt[h4][:, dc, tb2 * 512:(tb2 + 1) * 512],
                                    start=(dc == 0), stop=(dc == DC - 1))
                        rows = P if vc < NVC - 1 else VSH - (NVC - 1) * P
                        for tb2 in range(2):
                            ev = pfc.tile([P, 512], f32, tag="fc_ev",
                                          bufs=4, name=f"fcev{h4}_{vc}_{tb2}")
                            nc.vector.tensor_scalar_add(ev[:], psf[tb2][:],
                                                        fb_s[:, vc:vc + 1])
                            nc.scalar.dma_start(
                                out=outT[:][vc * P:vc * P + rows,
                                            h4 * 1024 + tb2 * 512:
                                            h4 * 1024 + (tb2 + 1) * 512],
                                in_=ev[:rows, :])

                fw_pre = {vc: load_fw(0, vc) for vc in range(6)}
                emit_rt(0)
                emit_fc(0, range(0, 8), fw_pre)
                emit_proj(1, psF, "fc")
                emit_fc(0, range(8, NVC))
                emit_rt(1)
                emit_fc(1, range(NVC))

        dram.release()
        pconst.release()

    nc.compile()
    return nc


def _get_nc():
    if "nc" not in _CACHE:
        _CACHE["nc"] = _build()
    return _CACHE["nc"]


def _arr_pdc(a):
    """[D, N] -> [128, DC*N] with row d = dc*128 + p."""
    d, n = a.shape
    return np.ascontiguousarray(
        a.reshape(DC, P, n).transpose(1, 0, 2).reshape(P, DC * n))


def kernel(token_ids, we, pe, Wq, Wk, Wv, proj_w, proj_b, fc_w, fc_b):
    import ml_dtypes
    from concourse.bass_utils import run_bass_kernel_spmd

    bf16 = ml_dtypes.bfloat16

    tok = np.asarray(token_ids).astype(np.int64)
    we = np.asarray(we, dtype=np.float32)
    pe = np.asarray(pe, dtype=np.float32)[:T]
    Wq = np.asarray(Wq, dtype=np.float32)
    Wk = np.asarray(Wk, dtype=np.float32)
    Wv = np.asarray(Wv, dtype=np.float32)
    proj_w = np.asarray(proj_w, dtype=np.float32)
    proj_b = np.asarray(proj_b, dtype=np.float32)
    fc_w = np.asarray(fc_w, dtype=np.float32)
    fc_b = np.asarray(fc_b, dtype=np.float32)

    x = we[tok] + pe                      # [T, D] f32, on host
    xT = np.ascontiguousarray(x.T)        # [D, T]
    xt_arr = _arr_pdc(xT)                 # [128, DC*T]

    scale = np.float32(1.0 / np.sqrt(HS))
    in_maps = []
    for i in range(NCORES):
        h0 = HPC * i
        wq_i = _arr_pdc(np.concatenate(
            [Wq[h0 + j] for j in range(HPC)], axis=1) * scale)
        wk_i = _arr_pdc(np.concatenate(
            [Wk[h0 + j] for j in range(HPC)], axis=1))
        wv_i = _arr_pdc(np.concatenate(
            [Wv[h0 + j] for j in range(HPC)], axis=1))
        pw_i = _arr_pdc(proj_w[:, DOS * i:DOS * (i + 1)]).astype(bf16)
        # core's x rows + proj bias, transposed: [256, T] -> [128, 2*T]
        xpb_i = (x[:, DOS * i:DOS * (i + 1)] +
                 proj_b[DOS * i:DOS * (i + 1)]).T
        xpb_i = np.ascontiguousarray(
            xpb_i.reshape(2, P, T).transpose(1, 0, 2).reshape(P, 2 * T))
        # fc shard padded 4000 -> 4096 vocab cols, chunked [vc][p][dc][j]
        fw_pad = np.zeros((D, NVC * P), dtype=np.float32)
        fw_pad[:, :VSH] = fc_w[:, VSH * i:VSH * (i + 1)]
        fw_i = np.ascontiguousarray(
            fw_pad.reshape(DC, P, NVC, P).transpose(2, 1, 0, 3)
            .reshape(NVC * P, DC * P)).astype(bf16)
        fb_pad = np.zeros(NVC * P, dtype=np.float32)
        fb_pad[:VSH] = fc_b[VSH * i:VSH * (i + 1)]
        fb_i = np.ascontiguousarray(fb_pad.reshape(NVC, P).T)
        in_maps.append({
            "xt": xt_arr, "wq": wq_i, "wk": wk_i, "wv": wv_i,
            "pw": pw_i, "xpb": xpb_i, "fw": fw_i, "fb": fb_i,
        })

    nc = _get_nc()
    trace = bool(int(os.environ.get("BASSKERNEL_TRACE", "0")))
    res = run_bass_kernel_spmd(nc, in_maps, core_ids=list(range(NCORES)),
                               trace=trace)
    if trace and res.exec_time_ns is not None:
        print(f"HW exec time: {res.exec_time_ns} ns")
        if res.instructions_and_trace is not None:
            print(f"Trace: {res.instructions_and_trace[1]}")

    out = np.empty((T, V), dtype=np.float32)
    for i in range(NCORES):
        out[:, VSH * i:VSH * (i + 1)] = res.results[i]["outT"].T
    return out
